# revision 17
# baseline (speedup 1.0000x reference)
"""FECAM layer Trainium2 kernel — v3: DCT folded into W1, two-GEMM device.

Reference (per batch element b, X = x[b] in R^{512x512}, layout [l, c]):
    freq = DCT-II(X^T along l)   [c, k]
    sd   = LN_k(freq)*gamma+beta
    h    = relu(sd @ W1^T); fw = sigmoid(h @ W2^T)
    fw   = LN_k(fw)*gamma+beta
    out  = X .* fw^T             [l, c]

Restructuring vs v2 (device DCT + folded basis):
  * The DCT AND the LN1 mean subtraction are linear in x, so both fold into
    W1 on the host:  mu1*rstd1 = sum_l xs[l,c]*dbar[l]/C  (dbar = column
    sums of the DCT basis), hence
        h_pre[h,c] = sum_l xs[l,c] * W1D'[h,l] + (W1@beta)[h],
        W1D' = (W1*gamma) @ (D - dbar/C),  xs = x * rstd1 (host Gram stats).
    Device fc1 consumes xs directly: the 6 DCT matmuls/batch (8.6% of PE),
    the mur upload + GPSIMD partition-broadcast, and the 4 DVE z-evictions
    per batch all disappear.  Host-emulated end-to-end rel err ~4.9e-3 vs
    the 2e-2 gate (better than v2's 8.8e-3 — fewer bf16 rounding steps).
  * Head: batch-0 fc1 runs lt-outer across all 8 PSUM banks so the first
    matmul needs only xs(0) plane 0 + w1d plane 0 (384KB) instead of the
    full 1.5MB first-batch working set; loads are split per-plane and
    interleaved across both hwdge queues (sync: xs0 planes, xs1, w2t lo;
    scalar: b1, w1d planes, w2t hi).  The act table load is emitted after
    the head DMA triggers so it doesn't delay them on the Scalar engine.
  * Tail: no gpsimd swdge stores anywhere (v2's final barrier waited ~9us
    on that slow queue draining batch-14 stores).  Steady-state stores ride
    the scalar hwdge queue (balanced against xs loads on sync); the last
    batch's stores split per-tile across sync+scalar.
  * Engine budget/batch: PE 32768 cyc (fc1 16384 + fc2 16384), ACT ~10k,
    DVE ~6.5k => PE-bound at ~13.7us/batch @2.4GHz, 16 batches/core
    => ~218us + head/tail vs v2's ~239us + 33us head/tail.
"""

import sys

if "/opt/trn_rl_repo" not in sys.path:
    sys.path.insert(0, "/opt/trn_rl_repo")

import numpy as np

P = 128
C = 512           # channels == seq len == dct size
H = 1024          # hidden
LT = 4            # l tiles (contraction for fc1)
HT = H // P       # 8 h-tiles
CT = C // P       # 4 c-tiles
EPS = 1e-6
N_CORES = 8
B_FULL = 128
MAGIC = 0x5F3759DF

_NC_CACHE: dict = {}


def _build(nb: int):
    import concourse.bass as bass
    from concourse import bacc
    import concourse.mybir as mybir
    from concourse.tile import TileContext

    f32 = mybir.dt.float32
    bf16 = mybir.dt.bfloat16
    i32 = mybir.dt.int32
    Relu = mybir.ActivationFunctionType.Relu
    Sigmoid = mybir.ActivationFunctionType.Sigmoid

    # all tensors stored partition-major in DRAM (host pre-shuffles) so
    # every DMA line is one 1-8KB contiguous chunk per partition
    nc = bacc.Bacc()
    xs_d = nc.declare_dram_parameter("xs", [nb, P, LT, C], bf16, isOutput=False)
    w1d_d = nc.declare_dram_parameter("w1d", [P, LT, H], bf16, isOutput=False)
    b1_d = nc.declare_dram_parameter("b1", [P, HT], f32, isOutput=False)
    w2t_d = nc.declare_dram_parameter("w2t", [P, HT, C], bf16, isOutput=False)
    out_d = nc.declare_dram_parameter("out", [nb, P, CT, C], bf16, isOutput=True)

    with TileContext(nc) as tc, \
            tc.tile_pool(name="consts", bufs=1) as consts, \
            tc.tile_pool(name="xin", bufs=4) as xin, \
            tc.tile_pool(name="hp", bufs=2) as hp, \
            tc.tile_pool(name="fwp", bufs=2) as fwp, \
            tc.tile_pool(name="resp", bufs=2) as resp, \
            tc.tile_pool(name="small", bufs=8) as small, \
            tc.tile_pool(name="ps_a", bufs=4, space="PSUM") as ps_a, \
            tc.tile_pool(name="ps_b", bufs=4, space="PSUM") as ps_b:

        w1d_sb = consts.tile([P, LT, H], bf16)
        w2t_sb = consts.tile([P, HT, C], bf16)
        b1_sb = consts.tile([P, HT], f32)
        magic_sb = consts.tile([P, CT], i32)
        warm_sb = consts.tile([P, 256], bf16)
        nc.vector.memset(magic_sb, MAGIC)
        nc.vector.memset(warm_sb, 0)

        st: dict = {}   # per-batch live tiles

        def emit_act_table():
            # single ACT table covering Sigmoid/Relu: pre-seeded (so the
            # availability pass never inserts another load) but AFTER the
            # head DMA triggers — the 1.3us load would otherwise delay the
            # w1d/b1 transfers on the Scalar engine's hwdge queue
            from concourse.hw_specs import get_activation_tables
            set_names = list(get_activation_tables(nc.m.arch))
            nc.scalar.add_instruction(mybir.InstLoadActFuncSet(
                name=nc.get_next_instruction_name(),
                act_func_set_id=set_names.index("sigmoid_and_others"),
                ins=[], outs=[]))

        def emit_load(b):
            x = xin.tile([P, LT, C], bf16, tag="xs")
            nc.sync.dma_start(out=x, in_=xs_d[b])
            st[b] = {"xs": x}

        def emit_fc1_b0():
            # batch 0 runs lt-outer so round lt starts as soon as xs0 plane
            # lt + w1d plane lt arrive — 8 simultaneous accumulating psum
            # banks (all of PSUM; fc2 hasn't started so ps_b is free).
            # Each round's load triggers are emitted immediately before its
            # matmuls (a matmul's DMA wait can cover every prior DMA on the
            # queue, so late loads must not be emitted early), and bytes are
            # balanced across the queues: scalar w1d+w2t.hi (1.54MB), sync
            # xs0+xs1+w2t.lo (1.54MB).
            x0 = xin.tile([P, LT, C], bf16, tag="xs")
            x1 = xin.tile([P, LT, C], bf16, tag="xs")
            st[0] = {"xs": x0}
            st[1] = {"xs": x1}
            hT = hp.tile([P, HT, C], bf16, tag="hT", name="hT")
            st[0]["hT"] = hT
            ps = [ps_a.tile([P, C], mybir.dt.float32, tag="ph",
                            name=f"ps0_{j}") for j in range(4)]
            ps += [ps_b.tile([P, C], mybir.dt.float32, tag="pw",
                             name=f"ps0_{j + 4}") for j in range(4)]
            # PE pstate warmup: the DVFS ramp (0.65 -> 2.4GHz over ~3us of
            # PE activity) would otherwise run on the first real matmuls
            # (observed 607ns vs 215ns steady); burn it on dummy 64x256
            # matmuls while the PE waits out the head DMA fill anyway
            warm_ps = ps_b.tile([P, C], mybir.dt.float32, tag="pw",
                                name="warm_ps")
            for _ in range(12):
                nc.tensor.matmul(warm_ps[0:64, 0:256], lhsT=warm_sb[:, 0:64],
                                 rhs=warm_sb[:, 0:256], start=True, stop=True)
            for lt in range(LT):
                nc.scalar.dma_start(out=w1d_sb[:, lt, :], in_=w1d_d[:, lt, :])
                nc.sync.dma_start(out=x0[:, lt, :], in_=xs_d[0, :, lt, :])
                if lt == 0:
                    # act table load on the Scalar engine AFTER the first
                    # w1d trigger: its 1.3us overlaps the w1d.p0 transfer
                    # instead of delaying the trigger
                    emit_act_table()
                if lt == 1:
                    nc.scalar.dma_start(out=b1_sb, in_=b1_d[:])
                for mh in range(HT):
                    nc.tensor.matmul(
                        ps[mh],
                        lhsT=w1d_sb[:, lt, mh * P:(mh + 1) * P],
                        rhs=x0[:, lt, :],
                        start=(lt == 0),
                        stop=(lt == LT - 1),
                    )
                    if lt == LT - 1:
                        nc.scalar.activation(out=hT[:, mh, :], in_=ps[mh],
                                             func=Relu,
                                             bias=b1_sb[:, mh:mh + 1],
                                             scale=1.0)
            nc.sync.dma_start(out=x1, in_=xs_d[1])
            nc.scalar.dma_start(out=w2t_sb[:, 4:8, :], in_=w2t_d[:, 4:8, :])
            nc.sync.dma_start(out=w2t_sb[:, 0:4, :], in_=w2t_d[:, 0:4, :])

        def emit_fc1(b, mh):
            if mh == 0:
                st[b]["hT"] = hp.tile([P, HT, C], bf16, tag="hT", name="hT")
            xsb = st[b]["xs"]
            hT = st[b]["hT"]
            ph = ps_a.tile([P, C], mybir.dt.float32, tag="ph")
            for lt in range(LT):
                nc.tensor.matmul(
                    ph,
                    lhsT=w1d_sb[:, lt, mh * P:(mh + 1) * P],
                    rhs=xsb[:, lt, :],
                    start=(lt == 0),
                    stop=(lt == LT - 1),
                )
            nc.scalar.activation(out=hT[:, mh, :], in_=ph, func=Relu,
                                 bias=b1_sb[:, mh:mh + 1], scale=1.0)
            if mh == HT - 1:
                del st[b]["xs"]

        def emit_fc2(b, mc):
            if mc == 0:
                st[b]["fw"] = fwp.tile([P, CT, C], bf16, tag="fw", name="fw")
                st[b]["mv"] = small.tile([P, CT, 2], mybir.dt.float32,
                                         tag="mv", name="mv")
            hT = st[b]["hT"]
            fw = st[b]["fw"]
            pw = ps_b.tile([P, C], mybir.dt.float32, tag="pw")
            for ht in range(HT):
                nc.tensor.matmul(
                    pw,
                    lhsT=hT[:, ht, mc * P:(mc + 1) * P],
                    rhs=w2t_sb[:, ht, :],
                    start=(ht == 0),
                    stop=(ht == HT - 1),
                )
            nc.scalar.activation(out=fw[:, mc, :], in_=pw, func=Sigmoid,
                                 bias=0.0, scale=1.0)
            stats = small.tile([P, 6], mybir.dt.float32, tag="stats")
            nc.vector.bn_stats(out=stats, in_=fw[:, mc, :])
            nc.vector.bn_aggr(out=st[b]["mv"][:, mc, :], in_=stats)
            if mc == CT - 1:
                del st[b]["hT"]

        def emit_ln2(b, mcs, dma_eng, split_stores=False):
            # LN2 apply for the given contiguous c-tiles, emitted right
            # after their fc2 stats so the last batch's tail overlaps the
            # remaining fc2 matmuls.
            f32_ = mybir.dt.float32
            i32_ = mybir.dt.int32
            mv = st[b]["mv"]
            if mcs[0] == 0:
                st[b]["u"] = small.tile([P, CT], f32_, tag="u", name="u")
                st[b]["y"] = small.tile([P, CT], f32_, tag="y", name="y")
                st[b]["t"] = small.tile([P, CT], f32_, tag="t", name="t")
                st[b]["res"] = resp.tile([P, CT, C], bf16, tag="res",
                                         name="res")
            u, y, t, res = (st[b][k] for k in ("u", "y", "t", "res"))
            sl = slice(mcs[0], mcs[-1] + 1)
            # rstd2 = rsqrt(var + eps): bit-trick seed + one Newton step
            # (seed err 3.4% -> rstd err ~1.7e-3, ~0.2% on the output).
            # (neuronxcc rejects DVE ALU opcodes on the Pool engine, so
            # this 7-op chain must stay on DVE.)
            ge = nc.vector
            ge.tensor_scalar_add(out=u[:, sl], in0=mv[:, sl, 1], scalar1=EPS)
            ge.tensor_scalar(out=y[:, sl].bitcast(i32_),
                             in0=u[:, sl].bitcast(i32_),
                             scalar1=1, scalar2=None,
                             op0=mybir.AluOpType.logical_shift_right)
            ge.tensor_tensor(out=y[:, sl].bitcast(i32_),
                             in0=magic_sb[:, sl],
                             in1=y[:, sl].bitcast(i32_),
                             op=mybir.AluOpType.subtract)
            ge.tensor_mul(out=t[:, sl], in0=u[:, sl], in1=y[:, sl])
            ge.tensor_mul(out=t[:, sl], in0=t[:, sl], in1=y[:, sl])
            ge.tensor_scalar(out=t[:, sl], in0=t[:, sl],
                             scalar1=-0.5, scalar2=1.5,
                             op0=mybir.AluOpType.mult,
                             op1=mybir.AluOpType.add)
            ge.tensor_mul(out=y[:, sl], in0=y[:, sl], in1=t[:, sl])
            fw = st[b]["fw"]
            for mc in mcs:
                # res = (fw - mu2) * rstd2  (4x-mode DVE tensor_scalar);
                # gamma/beta and the elementwise multiply by x^T happen on
                # the host, halving device DMA (no x^T upload)
                nc.vector.tensor_scalar(out=res[:, mc, :], in0=fw[:, mc, :],
                                        scalar1=mv[:, mc, 0:1],
                                        scalar2=y[:, mc:mc + 1],
                                        op0=mybir.AluOpType.subtract,
                                        op1=mybir.AluOpType.mult)
            if split_stores:
                # tail: one store per tile, alternating queues, so the two
                # final transfers drain in parallel
                engs = (nc.sync, nc.scalar)
                for j, mc in enumerate(mcs):
                    engs[j % 2].dma_start(out=out_d[b, :, mc:mc + 1, :],
                                          in_=res[:, mc:mc + 1, :])
            else:
                dma_eng.dma_start(out=out_d[b, :, sl, :], in_=res[:, sl, :])
            if mcs[-1] == CT - 1:
                del st[b]

        # software pipeline, 1-batch skew:
        #   cycle i: fc1(i) | fc2(i-1) + ln2(i-1) + stores
        # fc2(i-1)'s hT was fully evicted during fc1(i-1), a full cycle of
        # slack; fc1(i)'s xs was prefetched during cycle i-1.
        for i in range(nb + 1):
            if i == 0:
                emit_fc1_b0()
                continue
            # prefetch 2 batches ahead (bufs=4) so the xs-ready semaphore is
            # set well before fc1(i) reaches it
            if i == 1:
                emit_load(2)
            if 2 <= i + 2 < nb:
                emit_load(i + 2)
            if i < nb:
                for mh in range(HT):
                    emit_fc1(i, mh)
            for mc in range(CT):
                emit_fc2(i - 1, mc)
                if i == nb:
                    # last batch: one combined (2,3) chain (a split (2,)/(3,)
                    # chain pair is DVE-serial after the last matmul anyway
                    # and costs an extra ~1.2us); final stores split across
                    # both hwdge queues so the transfers drain in parallel
                    if mc == 1:
                        emit_ln2(i - 1, (0, 1), nc.sync)
                    elif mc == 3:
                        emit_ln2(i - 1, (2, 3), None, split_stores=True)
                else:
                    # steady state: stores on the scalar hwdge queue,
                    # balancing the xs loads on sync (~37GB/s each)
                    if mc == 1:
                        emit_ln2(i - 1, (0, 1), nc.scalar)
                    elif mc == 3:
                        emit_ln2(i - 1, (2, 3), nc.scalar)

    nc.finalize()
    return nc


def get_nc(nb: int):
    if nb not in _NC_CACHE:
        _NC_CACHE[nb] = _build(nb)
    return _NC_CACHE[nb]


def make_host_inputs(x, gamma, beta, w1, w2):
    """Host-side precompute: LN1 stats (Gram identity), DCT+LN1-mean fold
    into W1, weight layouts. All O(B*C^2) / O(H*C^2) passes."""
    import ml_dtypes
    bf = ml_dtypes.bfloat16

    x = np.ascontiguousarray(np.asarray(x, dtype=np.float32))
    gamma = np.asarray(gamma, dtype=np.float32)
    beta = np.asarray(beta, dtype=np.float32)
    w1 = np.asarray(w1, dtype=np.float32)
    w2 = np.asarray(w2, dtype=np.float32)

    k = np.arange(C)[:, None].astype(np.float64)
    m = np.arange(C)[None, :].astype(np.float64)
    D = 2.0 * np.cos(np.pi * k * (2.0 * m + 1.0) / (2.0 * C))    # [k, l]

    # LN1 stats from x via the DCT-II Gram identity D^T D = 2*ones + 2C*I
    xd = x.astype(np.float64)
    s = xd.sum(axis=1)                                  # [B, C] col sums
    q = np.einsum("blc,blc->bc", xd, xd, optimize=True)  # col sum-squares
    dbar = D.sum(axis=0)                                # [L]
    mu = np.einsum("l,blc->bc", dbar, xd, optimize=True) / C      # [B, C]
    var = (2.0 * s * s + 2.0 * C * q) / C - mu * mu
    rstd = 1.0 / np.sqrt(var + EPS)                     # [B, C]

    # fold rstd1 into x; partition-major [B, P, LT, C]
    xs = x * rstd[:, None, :].astype(np.float32)        # [B, L, C]
    xs = np.ascontiguousarray(
        xs.reshape(-1, LT, P, C).transpose(0, 2, 1, 3)).astype(bf)

    # DCT + LN1 mean removal + gamma folded into w1:
    #   W1D'[h,l] = sum_k w1[h,k]*gamma[k]*(D[k,l] - dbar[l]/C)
    Dp = D - dbar[None, :] / C
    W1Dp = (w1.astype(np.float64) * gamma.astype(np.float64)[None, :]) @ Dp
    w1d = np.ascontiguousarray(
        W1Dp.T.reshape(LT, P, H).transpose(1, 0, 2)).astype(bf)  # [P, LT, H]
    b1 = np.ascontiguousarray(
        (w1 @ beta).astype(np.float32).reshape(HT, P).T)         # [P, HT]
    w2t = np.ascontiguousarray(
        w2.T.reshape(HT, P, C).transpose(1, 0, 2)).astype(bf)    # [P, HT, C]

    const = dict(w1d=w1d, b1=b1, w2t=w2t)
    per_batch = dict(xs=xs)
    return per_batch, const


def make_in_maps(per_batch, const):
    nb = B_FULL // N_CORES
    return [
        {**{k: v[i * nb:(i + 1) * nb] for k, v in per_batch.items()}, **const}
        for i in range(N_CORES)
    ]


def postprocess(results, x, gamma, beta):
    """[n_cores] of {'out': z2n [nb, P, CT, L] bf16} -> full [B, L, C] fp32.

    The device returns LN2-normalized z2; the LN2 gamma/beta affine and the
    final elementwise multiply by x^T run here (O(B*C^2), ~0.02% of FLOPs).
    """
    out_p = np.concatenate([results[i]["out"] for i in range(N_CORES)], axis=0)
    B = out_p.shape[0]
    z2n = out_p.astype(np.float32).transpose(0, 2, 1, 3).reshape(B, C, C)
    fwln = z2n * gamma[None, None, :] + beta[None, None, :]   # [B, C(c), L(l)]
    x = np.asarray(x, dtype=np.float32)[:B]
    return np.ascontiguousarray(fwln.transpose(0, 2, 1) * x)


def kernel(x, gamma, beta, w1, w2):
    import time
    from concourse.bass_utils import run_bass_kernel_spmd

    per_batch, const = make_host_inputs(x, gamma, beta, w1, w2)
    nc = get_nc(B_FULL // N_CORES)
    in_maps = make_in_maps(per_batch, const)
    last_err = None
    for attempt in range(3):
        try:
            r = run_bass_kernel_spmd(nc, in_maps, list(range(N_CORES)))
            return postprocess(r.results, x, gamma, beta)
        except Exception as e:  # transient device wedge recovers on retry
            last_err = e
            time.sleep(5)
    raise last_err


# revision 20
# speedup vs baseline: 1.0027x; 1.0027x over previous
"""FECAM layer Trainium2 kernel — v3: DCT folded into W1, two-GEMM device.

Reference (per batch element b, X = x[b] in R^{512x512}, layout [l, c]):
    freq = DCT-II(X^T along l)   [c, k]
    sd   = LN_k(freq)*gamma+beta
    h    = relu(sd @ W1^T); fw = sigmoid(h @ W2^T)
    fw   = LN_k(fw)*gamma+beta
    out  = X .* fw^T             [l, c]

Restructuring vs v2 (device DCT + folded basis):
  * The DCT AND the LN1 mean subtraction are linear in x, so both fold into
    W1 on the host:  mu1*rstd1 = sum_l xs[l,c]*dbar[l]/C  (dbar = column
    sums of the DCT basis), hence
        h_pre[h,c] = sum_l xs[l,c] * W1D'[h,l] + (W1@beta)[h],
        W1D' = (W1*gamma) @ (D - dbar/C),  xs = x * rstd1 (host Gram stats).
    Device fc1 consumes xs directly: the 6 DCT matmuls/batch (8.6% of PE),
    the mur upload + GPSIMD partition-broadcast, and the 4 DVE z-evictions
    per batch all disappear.  Host-emulated end-to-end rel err ~4.9e-3 vs
    the 2e-2 gate (better than v2's 8.8e-3 — fewer bf16 rounding steps).
  * Head: batch-0 fc1 runs lt-outer across all 8 PSUM banks so the first
    matmul needs only xs(0) plane 0 + w1d plane 0 (384KB) instead of the
    full 1.5MB first-batch working set; loads are split per-plane and
    interleaved across both hwdge queues (sync: xs0 planes, xs1, w2t lo;
    scalar: b1, w1d planes, w2t hi).  The act table load is emitted after
    the head DMA triggers so it doesn't delay them on the Scalar engine.
  * Tail: no gpsimd swdge stores anywhere (v2's final barrier waited ~9us
    on that slow queue draining batch-14 stores).  Steady-state stores ride
    the scalar hwdge queue (balanced against xs loads on sync); the last
    batch's stores split per-tile across sync+scalar.
  * Engine budget/batch: PE 32768 cyc (fc1 16384 + fc2 16384), ACT ~10k,
    DVE ~6.5k => PE-bound at ~13.7us/batch @2.4GHz, 16 batches/core
    => ~218us + head/tail vs v2's ~239us + 33us head/tail.
"""

import sys

if "/opt/trn_rl_repo" not in sys.path:
    sys.path.insert(0, "/opt/trn_rl_repo")

import numpy as np

P = 128
C = 512           # channels == seq len == dct size
H = 1024          # hidden
LT = 4            # l tiles (contraction for fc1)
HT = H // P       # 8 h-tiles
CT = C // P       # 4 c-tiles
EPS = 1e-6
N_CORES = 8
B_FULL = 128
MAGIC = 0x5F3759DF

_NC_CACHE: dict = {}


def _build(nb: int):
    import concourse.bass as bass
    from concourse import bacc
    import concourse.mybir as mybir
    from concourse.tile import TileContext

    f32 = mybir.dt.float32
    bf16 = mybir.dt.bfloat16
    i32 = mybir.dt.int32
    Relu = mybir.ActivationFunctionType.Relu
    Sigmoid = mybir.ActivationFunctionType.Sigmoid

    # all tensors stored partition-major in DRAM (host pre-shuffles) so
    # every DMA line is one 1-8KB contiguous chunk per partition
    nc = bacc.Bacc()
    xs_d = nc.declare_dram_parameter("xs", [nb, P, LT, C], bf16, isOutput=False)
    w1d_d = nc.declare_dram_parameter("w1d", [P, LT, H], bf16, isOutput=False)
    b1_d = nc.declare_dram_parameter("b1", [P, HT], f32, isOutput=False)
    w2t_d = nc.declare_dram_parameter("w2t", [P, HT, C], bf16, isOutput=False)
    out_d = nc.declare_dram_parameter("out", [nb, P, CT, C], bf16, isOutput=True)

    with TileContext(nc) as tc, \
            tc.tile_pool(name="consts", bufs=1) as consts, \
            tc.tile_pool(name="xin", bufs=4) as xin, \
            tc.tile_pool(name="hp", bufs=2) as hp, \
            tc.tile_pool(name="fwp", bufs=2) as fwp, \
            tc.tile_pool(name="resp", bufs=2) as resp, \
            tc.tile_pool(name="small", bufs=8) as small, \
            tc.tile_pool(name="ps_a", bufs=4, space="PSUM") as ps_a, \
            tc.tile_pool(name="ps_b", bufs=4, space="PSUM") as ps_b:

        w1d_sb = consts.tile([P, LT, H], bf16)
        w2t_sb = consts.tile([P, HT, C], bf16)
        b1_sb = consts.tile([P, HT], f32)
        magic_sb = consts.tile([P, CT], i32)
        warm_sb = consts.tile([P, 256], bf16)
        nc.vector.memset(magic_sb, MAGIC)
        nc.vector.memset(warm_sb, 0)

        st: dict = {}   # per-batch live tiles

        def emit_act_table():
            # single ACT table covering Sigmoid/Relu: pre-seeded (so the
            # availability pass never inserts another load) but AFTER the
            # head DMA triggers — the 1.3us load would otherwise delay the
            # w1d/b1 transfers on the Scalar engine's hwdge queue
            from concourse.hw_specs import get_activation_tables
            set_names = list(get_activation_tables(nc.m.arch))
            nc.scalar.add_instruction(mybir.InstLoadActFuncSet(
                name=nc.get_next_instruction_name(),
                act_func_set_id=set_names.index("sigmoid_and_others"),
                ins=[], outs=[]))

        def emit_load(b):
            x = xin.tile([P, LT, C], bf16, tag="xs")
            nc.sync.dma_start(out=x, in_=xs_d[b])
            st[b] = {"xs": x}

        def emit_fc1_b0():
            # batch 0 runs lt-outer so round lt starts as soon as xs0 plane
            # lt + w1d plane lt arrive — 8 simultaneous accumulating psum
            # banks (all of PSUM; fc2 hasn't started so ps_b is free).
            # Each round's load triggers are emitted immediately before its
            # matmuls (a matmul's DMA wait can cover every prior DMA on the
            # queue, so late loads must not be emitted early), and bytes are
            # balanced across the queues: scalar w1d+w2t.hi (1.54MB), sync
            # xs0+xs1+w2t.lo (1.54MB).
            x0 = xin.tile([P, LT, C], bf16, tag="xs")
            x1 = xin.tile([P, LT, C], bf16, tag="xs")
            st[0] = {"xs": x0}
            st[1] = {"xs": x1}
            hT = hp.tile([P, HT, C], bf16, tag="hT", name="hT")
            st[0]["hT"] = hT
            ps = [ps_a.tile([P, C], mybir.dt.float32, tag="ph",
                            name=f"ps0_{j}") for j in range(4)]
            ps += [ps_b.tile([P, C], mybir.dt.float32, tag="pw",
                             name=f"ps0_{j + 4}") for j in range(4)]
            # PE pstate warmup: the DVFS ramp (0.65 -> 2.4GHz over ~3us of
            # PE activity) would otherwise run on the first real matmuls
            # (observed 607ns vs 215ns steady); burn it on dummy 64x256
            # matmuls while the PE waits out the head DMA fill anyway
            warm_ps = ps_b.tile([P, C], mybir.dt.float32, tag="pw",
                                name="warm_ps")
            for _ in range(12):
                nc.tensor.matmul(warm_ps[0:64, 0:256], lhsT=warm_sb[:, 0:64],
                                 rhs=warm_sb[:, 0:256], start=True, stop=True)
            for lt in range(LT):
                nc.scalar.dma_start(out=w1d_sb[:, lt, :], in_=w1d_d[:, lt, :])
                nc.sync.dma_start(out=x0[:, lt, :], in_=xs_d[0, :, lt, :])
                if lt == 0:
                    # act table load on the Scalar engine AFTER the first
                    # w1d trigger: its 1.3us overlaps the w1d.p0 transfer
                    # instead of delaying the trigger
                    emit_act_table()
                if lt == 1:
                    nc.scalar.dma_start(out=b1_sb, in_=b1_d[:])
                for mh in range(HT):
                    nc.tensor.matmul(
                        ps[mh],
                        lhsT=w1d_sb[:, lt, mh * P:(mh + 1) * P],
                        rhs=x0[:, lt, :],
                        start=(lt == 0),
                        stop=(lt == LT - 1),
                    )
                    if lt == LT - 1:
                        nc.scalar.activation(out=hT[:, mh, :], in_=ps[mh],
                                             func=Relu,
                                             bias=b1_sb[:, mh:mh + 1],
                                             scale=1.0)
            nc.sync.dma_start(out=x1, in_=xs_d[1])
            nc.scalar.dma_start(out=w2t_sb[:, 4:8, :], in_=w2t_d[:, 4:8, :])
            nc.sync.dma_start(out=w2t_sb[:, 0:4, :], in_=w2t_d[:, 0:4, :])

        def emit_fc1(b, mh):
            if mh == 0:
                st[b]["hT"] = hp.tile([P, HT, C], bf16, tag="hT", name="hT")
            xsb = st[b]["xs"]
            hT = st[b]["hT"]
            ph = ps_a.tile([P, C], mybir.dt.float32, tag="ph")
            for lt in range(LT):
                nc.tensor.matmul(
                    ph,
                    lhsT=w1d_sb[:, lt, mh * P:(mh + 1) * P],
                    rhs=xsb[:, lt, :],
                    start=(lt == 0),
                    stop=(lt == LT - 1),
                )
            nc.scalar.activation(out=hT[:, mh, :], in_=ph, func=Relu,
                                 bias=b1_sb[:, mh:mh + 1], scale=1.0)
            if mh == HT - 1:
                del st[b]["xs"]

        def emit_fc2(b, mc):
            if mc == 0:
                st[b]["fw"] = fwp.tile([P, CT, C], bf16, tag="fw", name="fw")
                st[b]["mv"] = small.tile([P, CT, 2], mybir.dt.float32,
                                         tag="mv", name="mv")
            hT = st[b]["hT"]
            fw = st[b]["fw"]
            pw = ps_b.tile([P, C], mybir.dt.float32, tag="pw")
            for ht in range(HT):
                nc.tensor.matmul(
                    pw,
                    lhsT=hT[:, ht, mc * P:(mc + 1) * P],
                    rhs=w2t_sb[:, ht, :],
                    start=(ht == 0),
                    stop=(ht == HT - 1),
                )
            nc.scalar.activation(out=fw[:, mc, :], in_=pw, func=Sigmoid,
                                 bias=0.0, scale=1.0)
            stats = small.tile([P, 6], mybir.dt.float32, tag="stats")
            nc.vector.bn_stats(out=stats, in_=fw[:, mc, :])
            nc.vector.bn_aggr(out=st[b]["mv"][:, mc, :], in_=stats)
            if mc == CT - 1:
                del st[b]["hT"]

        def emit_fc2_last_halves(b):
            # last batch, last c-tile: run fc2 as two k-halves so the first
            # half's sigmoid+stats hide behind the second half's matmuls —
            # only the half-sized sigmoid/stats remain on the exposed tail.
            # bn_aggr combines the two half-records (equal counts) exactly.
            hT = st[b]["hT"]
            fw = st[b]["fw"]
            pw = ps_b.tile([P, C], mybir.dt.float32, tag="pw")
            both = small.tile([P, 12], mybir.dt.float32, tag="stats",
                              name="stats_hh")
            for half in range(2):
                sl = slice(half * 256, (half + 1) * 256)
                for ht in range(HT):
                    nc.tensor.matmul(
                        pw[:, sl],
                        lhsT=hT[:, ht, 3 * P:4 * P],
                        rhs=w2t_sb[:, ht, sl],
                        start=(ht == 0),
                        stop=(ht == HT - 1),
                    )
                nc.scalar.activation(out=fw[:, 3, sl], in_=pw[:, sl],
                                     func=Sigmoid, bias=0.0, scale=1.0)
                nc.vector.bn_stats(out=both[:, 6 * half:6 * half + 6],
                                   in_=fw[:, 3, sl])
            nc.vector.bn_aggr(out=st[b]["mv"][:, 3, :], in_=both)
            del st[b]["hT"]

        def emit_ln2(b, mcs, dma_eng, split_stores=False):
            # LN2 apply for the given contiguous c-tiles, emitted right
            # after their fc2 stats so the last batch's tail overlaps the
            # remaining fc2 matmuls.
            f32_ = mybir.dt.float32
            i32_ = mybir.dt.int32
            mv = st[b]["mv"]
            if mcs[0] == 0:
                st[b]["u"] = small.tile([P, CT], f32_, tag="u", name="u")
                st[b]["y"] = small.tile([P, CT], f32_, tag="y", name="y")
                st[b]["t"] = small.tile([P, CT], f32_, tag="t", name="t")
                st[b]["res"] = resp.tile([P, CT, C], bf16, tag="res",
                                         name="res")
            u, y, t, res = (st[b][k] for k in ("u", "y", "t", "res"))
            sl = slice(mcs[0], mcs[-1] + 1)
            # rstd2 = rsqrt(var + eps): bit-trick seed + one Newton step
            # (seed err 3.4% -> rstd err ~1.7e-3, ~0.2% on the output).
            # (neuronxcc rejects DVE ALU opcodes on the Pool engine, so
            # this 7-op chain must stay on DVE.)
            ge = nc.vector
            ge.tensor_scalar_add(out=u[:, sl], in0=mv[:, sl, 1], scalar1=EPS)
            ge.tensor_scalar(out=y[:, sl].bitcast(i32_),
                             in0=u[:, sl].bitcast(i32_),
                             scalar1=1, scalar2=None,
                             op0=mybir.AluOpType.logical_shift_right)
            ge.tensor_tensor(out=y[:, sl].bitcast(i32_),
                             in0=magic_sb[:, sl],
                             in1=y[:, sl].bitcast(i32_),
                             op=mybir.AluOpType.subtract)
            ge.tensor_mul(out=t[:, sl], in0=u[:, sl], in1=y[:, sl])
            ge.tensor_mul(out=t[:, sl], in0=t[:, sl], in1=y[:, sl])
            ge.tensor_scalar(out=t[:, sl], in0=t[:, sl],
                             scalar1=-0.5, scalar2=1.5,
                             op0=mybir.AluOpType.mult,
                             op1=mybir.AluOpType.add)
            ge.tensor_mul(out=y[:, sl], in0=y[:, sl], in1=t[:, sl])
            fw = st[b]["fw"]
            for mc in mcs:
                # res = (fw - mu2) * rstd2  (4x-mode DVE tensor_scalar);
                # gamma/beta and the elementwise multiply by x^T happen on
                # the host, halving device DMA (no x^T upload)
                nc.vector.tensor_scalar(out=res[:, mc, :], in0=fw[:, mc, :],
                                        scalar1=mv[:, mc, 0:1],
                                        scalar2=y[:, mc:mc + 1],
                                        op0=mybir.AluOpType.subtract,
                                        op1=mybir.AluOpType.mult)
            if split_stores:
                # tail: one store per tile, alternating queues, so the two
                # final transfers drain in parallel
                engs = (nc.sync, nc.scalar)
                for j, mc in enumerate(mcs):
                    engs[j % 2].dma_start(out=out_d[b, :, mc:mc + 1, :],
                                          in_=res[:, mc:mc + 1, :])
            else:
                dma_eng.dma_start(out=out_d[b, :, sl, :], in_=res[:, sl, :])
            if mcs[-1] == CT - 1:
                del st[b]

        # software pipeline, 1-batch skew:
        #   cycle i: fc1(i) | fc2(i-1) + ln2(i-1) + stores
        # fc2(i-1)'s hT was fully evicted during fc1(i-1), a full cycle of
        # slack; fc1(i)'s xs was prefetched during cycle i-1.
        for i in range(nb + 1):
            if i == 0:
                emit_fc1_b0()
                continue
            # prefetch 2 batches ahead (bufs=4) so the xs-ready semaphore is
            # set well before fc1(i) reaches it
            if i == 1:
                emit_load(2)
            if 2 <= i + 2 < nb:
                emit_load(i + 2)
            if i < nb:
                for mh in range(HT):
                    emit_fc1(i, mh)
            for mc in range(CT):
                if i == nb and mc == CT - 1:
                    emit_fc2_last_halves(i - 1)
                else:
                    emit_fc2(i - 1, mc)
                if i == nb:
                    # last batch: one combined (2,3) chain (a split (2,)/(3,)
                    # chain pair is DVE-serial after the last matmul anyway
                    # and costs an extra ~1.2us); final stores split across
                    # both hwdge queues so the transfers drain in parallel
                    if mc == 1:
                        emit_ln2(i - 1, (0, 1), nc.sync)
                    elif mc == 3:
                        emit_ln2(i - 1, (2, 3), None, split_stores=True)
                else:
                    # steady state: stores on the scalar hwdge queue,
                    # balancing the xs loads on sync (~37GB/s each)
                    if mc == 1:
                        emit_ln2(i - 1, (0, 1), nc.scalar)
                    elif mc == 3:
                        emit_ln2(i - 1, (2, 3), nc.scalar)

    nc.finalize()
    return nc


def get_nc(nb: int):
    if nb not in _NC_CACHE:
        _NC_CACHE[nb] = _build(nb)
    return _NC_CACHE[nb]


def make_host_inputs(x, gamma, beta, w1, w2):
    """Host-side precompute: LN1 stats (Gram identity), DCT+LN1-mean fold
    into W1, weight layouts. All O(B*C^2) / O(H*C^2) passes."""
    import ml_dtypes
    bf = ml_dtypes.bfloat16

    x = np.ascontiguousarray(np.asarray(x, dtype=np.float32))
    gamma = np.asarray(gamma, dtype=np.float32)
    beta = np.asarray(beta, dtype=np.float32)
    w1 = np.asarray(w1, dtype=np.float32)
    w2 = np.asarray(w2, dtype=np.float32)

    k = np.arange(C)[:, None].astype(np.float64)
    m = np.arange(C)[None, :].astype(np.float64)
    D = 2.0 * np.cos(np.pi * k * (2.0 * m + 1.0) / (2.0 * C))    # [k, l]

    # LN1 stats from x via the DCT-II Gram identity D^T D = 2*ones + 2C*I
    xd = x.astype(np.float64)
    s = xd.sum(axis=1)                                  # [B, C] col sums
    q = np.einsum("blc,blc->bc", xd, xd, optimize=True)  # col sum-squares
    dbar = D.sum(axis=0)                                # [L]
    mu = np.einsum("l,blc->bc", dbar, xd, optimize=True) / C      # [B, C]
    var = (2.0 * s * s + 2.0 * C * q) / C - mu * mu
    rstd = 1.0 / np.sqrt(var + EPS)                     # [B, C]

    # fold rstd1 into x; partition-major [B, P, LT, C]
    xs = x * rstd[:, None, :].astype(np.float32)        # [B, L, C]
    xs = np.ascontiguousarray(
        xs.reshape(-1, LT, P, C).transpose(0, 2, 1, 3)).astype(bf)

    # DCT + LN1 mean removal + gamma folded into w1:
    #   W1D'[h,l] = sum_k w1[h,k]*gamma[k]*(D[k,l] - dbar[l]/C)
    Dp = D - dbar[None, :] / C
    W1Dp = (w1.astype(np.float64) * gamma.astype(np.float64)[None, :]) @ Dp
    w1d = np.ascontiguousarray(
        W1Dp.T.reshape(LT, P, H).transpose(1, 0, 2)).astype(bf)  # [P, LT, H]
    b1 = np.ascontiguousarray(
        (w1 @ beta).astype(np.float32).reshape(HT, P).T)         # [P, HT]
    w2t = np.ascontiguousarray(
        w2.T.reshape(HT, P, C).transpose(1, 0, 2)).astype(bf)    # [P, HT, C]

    const = dict(w1d=w1d, b1=b1, w2t=w2t)
    per_batch = dict(xs=xs)
    return per_batch, const


def make_in_maps(per_batch, const):
    nb = B_FULL // N_CORES
    return [
        {**{k: v[i * nb:(i + 1) * nb] for k, v in per_batch.items()}, **const}
        for i in range(N_CORES)
    ]


def postprocess(results, x, gamma, beta):
    """[n_cores] of {'out': z2n [nb, P, CT, L] bf16} -> full [B, L, C] fp32.

    The device returns LN2-normalized z2; the LN2 gamma/beta affine and the
    final elementwise multiply by x^T run here (O(B*C^2), ~0.02% of FLOPs).
    """
    out_p = np.concatenate([results[i]["out"] for i in range(N_CORES)], axis=0)
    B = out_p.shape[0]
    z2n = out_p.astype(np.float32).transpose(0, 2, 1, 3).reshape(B, C, C)
    fwln = z2n * gamma[None, None, :] + beta[None, None, :]   # [B, C(c), L(l)]
    x = np.asarray(x, dtype=np.float32)[:B]
    return np.ascontiguousarray(fwln.transpose(0, 2, 1) * x)


def kernel(x, gamma, beta, w1, w2):
    import time
    from concourse.bass_utils import run_bass_kernel_spmd

    per_batch, const = make_host_inputs(x, gamma, beta, w1, w2)
    nc = get_nc(B_FULL // N_CORES)
    in_maps = make_in_maps(per_batch, const)
    last_err = None
    for attempt in range(3):
        try:
            r = run_bass_kernel_spmd(nc, in_maps, list(range(N_CORES)))
            return postprocess(r.results, x, gamma, beta)
        except Exception as e:  # transient device wedge recovers on retry
            last_err = e
            time.sleep(5)
    raise last_err
